# revision 2
# baseline (speedup 1.0000x reference)
"""Fused 2D-RoPE multi-head attention for Trainium2, SPMD over 8 NeuronCores.

Problem: x[2,4,24,24,1024] -> fused QKV -> 16-head attention with 2-axis RoPE
-> out proj.  Data-parallel: one (b t) sequence (S=576, D=1024) per core.

v2 redesign (vs the first working kernel):
  - q/k weight rows are host-permuted into evens-tile/odds-tile PAIRS covering
    4 heads each (tile A = dims {2j} of heads 4g..4g+3, tile B = dims {2j+1}).
    The RoPE rotate-half then pairs partition p of tile A with partition p of
    tile B, so all RoPE arithmetic runs as full-width [128,576] ops (engine
    cost is free-size only; the old layout wasted 3/4 of every lane).
  - RoPE reads the QK-projection PSUM directly (no staging copies through
    the scalar engine); scalar engine only runs exp.
  - Each projection PSUM tile is [128,1024] f32 (2 banks) holding both
    sq-halves at offsets 0/512; exp / reciprocal / normalize / staging then
    process both halves in ONE instruction via a strided AP, halving
    instruction-count on the ACT/DVE engines.
  - Scores contract per head over 32 evens + 32 odds rows (2 accumulating
    matmuls); the 4 heads of a group sit on disjoint 32-row groups of the PE
    array, so their matmuls overlap in the array (row-group tiling).
  - E and the v-with-ones-column table are bf16 (same PE throughput, half
    SBUF/DVE bytes); everything else f32r.
  - Output projection emits [e,s] (oT is already the needed moving operand);
    the host transposes. All weights/inputs are host-packed in exact SBUF
    layout so every DMA is >=0.6MB fully-coalesced (18 DMAs/rep vs 79).
  - Denominator rides as a 65th ones-column in the att@v stationary operand;
    softmax has no max-subtraction (scores ~ N(0,1), exp is safe).
"""

import numpy as np
from contextlib import ExitStack

B, T, HH, WW, D = 2, 4, 24, 24, 1024
NH, HD = 16, 64
S = HH * WW            # 576
BT = B * T             # 8
NCORES = 8
P = 128
SQH = 288              # half of S
NKD = D // P           # 8 contraction tiles over D
S_TILES = [(0, 128), (128, 128), (256, 128), (384, 128), (512, 64)]
VSLOT = HD + 1         # 65: per-head v columns + ones column

_CACHE: dict = {}


def _cs_table():
    """cos/sin tables [128, S]: row 32*i + j = pair-frequency j (any head)."""
    half = HD // 4     # 16
    inv = (1.0 / (10000.0 ** (np.arange(half, dtype=np.float32) / np.float32(half)))).astype(np.float32)
    s = np.arange(S, dtype=np.float32)
    th = (s // WW)[None, :] * inv[:, None]          # [16, S]
    tw = (s % WW)[None, :] * inv[:, None]           # [16, S]
    ang = np.concatenate([th, tw], axis=0)          # [32, S]
    cos32 = np.cos(ang).astype(np.float32)
    sin32 = np.sin(ang).astype(np.float32)
    cos_t = np.tile(cos32, (4, 1))                  # [128, S]
    sin_t = np.tile(sin32, (4, 1))
    return np.ascontiguousarray(np.concatenate([cos_t, sin_t], axis=1))  # [128, 2S]


def _qk_rowperm():
    """Order of the 2048 q+k rows of w_qkv into 16 tiles of 128.

    Tile tt = 4g + r: r=0 kA (k evens), r=1 kB (k odds), r=2 qA, r=3 qB.
    Within a tile, col 32*i + j = head 4g+i, rotation pair j.
    """
    rows = []
    for g in range(4):
        for base, odd in ((D, 0), (D, 1), (0, 0), (0, 1)):
            for i in range(4):
                for j in range(32):
                    rows.append(base + (4 * g + i) * HD + 2 * j + odd)
    return np.asarray(rows)


def _build_nc(repeat=1):
    import concourse.bacc as bacc
    import concourse.mybir as mybir
    from concourse.tile import TileContext

    f32 = mybir.dt.float32
    f32r = mybir.dt.float32r
    bf16 = mybir.dt.bfloat16
    AF = mybir.ActivationFunctionType

    nc = bacc.Bacc("TRN2", target_bir_lowering=False, debug=False)
    xsb_d = nc.dram_tensor("xsb", [P, NKD * S], f32r, kind="ExternalInput").ap()
    wqk_d = nc.dram_tensor("wqk", [P, 16 * NKD * P], f32r, kind="ExternalInput").ap()
    wv_d = nc.dram_tensor("wv", [P, 4 * NKD * 256], f32r, kind="ExternalInput").ap()
    wo_d = nc.dram_tensor("wo", [P, 4 * 2048], f32r, kind="ExternalInput").ap()
    cs_d = nc.dram_tensor("cs", [P, 2 * S], f32, kind="ExternalInput").ap()
    out_d = nc.dram_tensor("out", [D, S], f32, kind="ExternalOutput").ap()

    def half2(ap):
        # [p, 1024] psum view -> [p, 2, 288] (the written halves at 0 / 512)
        return ap.rearrange("p (c x) -> p c x", x=512)[:, :, 0:SQH]

    def c2(ap):
        # [p, 576] sbuf view -> [p, 2, 288]
        return ap.rearrange("p (c x) -> p c x", c=2)

    with TileContext(nc) as tc, ExitStack() as ctx:
        const = ctx.enter_context(tc.tile_pool(name="const", bufs=1))
        wqkp = ctx.enter_context(tc.tile_pool(name="wqkp", bufs=2))
        wop = ctx.enter_context(tc.tile_pool(name="wop", bufs=2))
        tmpp = ctx.enter_context(tc.tile_pool(name="tmpp", bufs=4))
        ep = ctx.enter_context(tc.tile_pool(name="ep", bufs=8))
        r1p = ctx.enter_context(tc.tile_pool(name="r1p", bufs=2))
        rrp = ctx.enter_context(tc.tile_pool(name="rrp", bufs=2))
        stp = ctx.enter_context(tc.tile_pool(name="stp", bufs=2))
        psum = ctx.enter_context(tc.tile_pool(name="psum", bufs=4, space="PSUM"))

        xsb = const.tile([P, NKD * S], f32r, name="xsb_t")
        wv = const.tile([P, 4 * NKD * 256], f32r, name="wv_t")
        cs = const.tile([P, 2 * S], f32, name="cs_t")
        roped = [const.tile([P, 4 * S], f32r, name=f"roped{g}") for g in range(4)]
        va = const.tile([P, 5 * NH * VSLOT], bf16, name="va_t")
        vav = va.rearrange("p (st h c) -> p st h c", st=5, c=VSLOT)
        oT = [const.tile([P, 4 * S], f32r, name=f"oT{x}") for x in range(2)]

        nc.sync.dma_start(cs, cs_d)
        nc.gpsimd.memset(vav[:, :, :, HD:HD + 1], 1.0)

        cos_s = c2(cs[:, 0:S])
        sin_s = c2(cs[:, S:2 * S])
        xv = xsb.rearrange("p (k s) -> p k s", k=NKD)
        wvv = wv.rearrange("p (c k e) -> p c k e", c=4, e=256)

        for _rep in range(repeat):
            nc.sync.dma_start(xsb, xsb_d)
            nc.sync.dma_start(wv, wv_d)

            # ---- v projection: psv[s, e], one accumulation group per psum bank
            for ch in range(2):
                for st, (s0, sl) in enumerate(S_TILES):
                    psv = psum.tile([P, 1024], f32, tag="ps", name="psv")
                    for kt in range(NKD):
                        for cc in range(2):
                            nc.tensor.matmul(psv[0:sl, cc * 512:cc * 512 + 256],
                                             xv[:, kt, s0:s0 + sl],
                                             wvv[:, 2 * ch + cc, kt, :],
                                             start=(kt == 0), stop=(kt == NKD - 1))
                    src = psv.rearrange("p (b c h x) -> p b c h x",
                                        b=1, c=2, h=8)[0:sl, :, :, 0:4, :]
                    dst = va.rearrange("p (st c2 h4 x) -> p st c2 h4 x",
                                       st=5, c2=4, x=VSLOT)[0:sl, st:st + 1,
                                                            2 * ch:2 * ch + 2, :, 0:HD]
                    nc.vector.tensor_copy(dst, src)

            # ---- q,k projection + RoPE, per 4-head group g
            for g in range(4):
                for half in range(2):          # 0: k pair (A,B), 1: q pair
                    wt = wqkp.tile([P, 2 * NKD * P], f32r, name="wt")
                    nc.sync.dma_start(wt, wqk_d[:, (2 * g + half) * 2048:(2 * g + half + 1) * 2048])
                    psA = psum.tile([P, 1024], f32, tag="ps", name="psA")
                    psB = psum.tile([P, 1024], f32, tag="ps", name="psB")
                    for kt in range(NKD):
                        for ti, ps in ((0, psA), (1, psB)):
                            w_r = wt[:, (ti * NKD + kt) * P:(ti * NKD + kt + 1) * P]
                            for hf in range(2):
                                nc.tensor.matmul(ps[:, hf * 512:hf * 512 + SQH],
                                                 w_r, xv[:, kt, hf * SQH:(hf + 1) * SQH],
                                                 start=(kt == 0), stop=(kt == NKD - 1))
                    rA = roped[g][:, (2 * half) * S:(2 * half) * S + S]
                    rB = roped[g][:, (2 * half + 1) * S:(2 * half + 1) * S + S]
                    tA = tmpp.tile([P, S], f32, name="tA")
                    tB = tmpp.tile([P, S], f32, name="tB")
                    nc.vector.tensor_mul(c2(rA), half2(psA), cos_s)
                    nc.vector.tensor_mul(c2(tB), half2(psB), sin_s)
                    nc.vector.tensor_mul(c2(rB), half2(psB), cos_s)
                    nc.vector.tensor_mul(c2(tA), half2(psA), sin_s)
                    nc.gpsimd.tensor_sub(rA, rA, tB)   # evens: a*cos - b*sin
                    nc.gpsimd.tensor_add(rB, rB, tA)   # odds:  b*cos + a*sin

            # ---- attention, per head
            for g in range(4):
                for i in range(4):
                    h = 4 * g + i
                    Es = []
                    for ci, (c0, cl) in enumerate(S_TILES):
                        pss = psum.tile([P, 1024], f32, tag="ps", name="pss")
                        kA = roped[g][32 * i:32 * i + 32, c0:c0 + cl]
                        kB = roped[g][32 * i:32 * i + 32, S + c0:S + c0 + cl]
                        for hf in range(2):
                            qA = roped[g][32 * i:32 * i + 32, 2 * S + hf * SQH:2 * S + (hf + 1) * SQH]
                            nc.tensor.matmul(pss[0:cl, hf * 512:hf * 512 + SQH],
                                             kA, qA, start=True, stop=False,
                                             tile_position=(32 * i, 0))
                        for hf in range(2):
                            qB = roped[g][32 * i:32 * i + 32, 3 * S + hf * SQH:3 * S + (hf + 1) * SQH]
                            nc.tensor.matmul(pss[0:cl, hf * 512:hf * 512 + SQH],
                                             kB, qB, start=False, stop=True,
                                             tile_position=(32 * i, 0))
                        E = ep.tile([P, S], bf16, name="E")
                        nc.scalar.activation(c2(E[0:cl, :]), half2(pss[0:cl, :]),
                                             AF.Exp, scale=0.125)
                        Es.append(E)
                    pso = psum.tile([P, 1024], f32, tag="ps", name="pso")
                    for hf in range(2):
                        for ci, (c0, cl) in enumerate(S_TILES):
                            nc.tensor.matmul(pso[0:VSLOT, hf * 512:hf * 512 + SQH],
                                             vav[0:cl, ci:ci + 1, h:h + 1, :],
                                             Es[ci][0:cl, hf * SQH:(hf + 1) * SQH],
                                             start=(ci == 0), stop=(ci == 4))
                    r1 = r1p.tile([1, S], f32, name="r1")
                    nc.vector.reciprocal(c2(r1), half2(pso[HD:HD + 1, :]))
                    rr = rrp.tile([HD, S], f32, name="rr")
                    nc.gpsimd.partition_broadcast(rr, r1)
                    ot_dst = oT[h // 8][64 * (h % 2):64 * (h % 2) + 64,
                                        ((h // 2) % 4) * S:((h // 2) % 4) * S + S]
                    nc.vector.tensor_mul(
                        ot_dst.rearrange("p (b c x) -> p b c x", b=1, x=SQH),
                        pso.rearrange("p (b c x) -> p b c x", b=1, x=512)[0:HD, :, :, 0:SQH],
                        rr.rearrange("p (b c x) -> p b c x", b=1, x=SQH))

            # ---- output projection: out[e, s]
            for epi in range(4):
                wot = wop.tile([P, 2048], f32r, name="wot")
                nc.sync.dma_start(wot, wo_d[:, epi * 2048:(epi + 1) * 2048])
                ob = stp.tile([P, 2 * S], f32, name="ob")
                for e2 in range(2):
                    pso2 = psum.tile([P, 1024], f32, tag="ps", name="pso2")
                    for kt in range(NKD):
                        w_r = wot[:, (kt * 2 + e2) * P:(kt * 2 + e2 + 1) * P]
                        mv = oT[kt // 4][:, (kt % 4) * S:(kt % 4) * S + S]
                        for hf in range(2):
                            nc.tensor.matmul(pso2[:, hf * 512:hf * 512 + SQH],
                                             w_r, mv[:, hf * SQH:(hf + 1) * SQH],
                                             start=(kt == 0), stop=(kt == NKD - 1))
                    nc.vector.tensor_copy(c2(ob[:, e2 * S:(e2 + 1) * S]), half2(pso2))
                nc.sync.dma_start(out_d.rearrange("(t p) s -> p t s", p=P)[:, 2 * epi:2 * epi + 2, :],
                                  ob.rearrange("p (t s) -> p t s", t=2))
    nc.compile()
    return nc


def _prep_inputs(x, w_qkv, w_out):
    x = np.asarray(x, dtype=np.float32)
    w_qkv = np.asarray(w_qkv, dtype=np.float32)
    w_out = np.asarray(w_out, dtype=np.float32)
    xr = x.reshape(BT, S, D)

    perm = _qk_rowperm()
    wq_full = w_qkv[perm]                                   # [2048, 1024]
    wqk_sb = np.ascontiguousarray(
        wq_full.reshape(16, 128, NKD, P).transpose(3, 0, 2, 1).reshape(P, -1))
    wv_arr = w_qkv[2 * D:3 * D]                             # [1024(e), 1024(d)]
    wv_sb = np.ascontiguousarray(
        wv_arr.reshape(4, 256, NKD, P).transpose(3, 0, 2, 1).reshape(P, -1))
    wo_sb = np.ascontiguousarray(
        w_out.reshape(4, 2, P, NKD, P).transpose(4, 0, 3, 1, 2).reshape(P, -1))
    cs = _cs_table()

    in_maps = []
    for i in range(NCORES):
        xsb = np.ascontiguousarray(
            xr[i].T.reshape(NKD, P, S).transpose(1, 0, 2).reshape(P, -1))
        in_maps.append({
            "xsb": xsb, "wqk": wqk_sb, "wv": wv_sb, "wo": wo_sb, "cs": cs,
        })
    return in_maps


def get_nc(repeat=1):
    key = f"nc{repeat}"
    if key not in _CACHE:
        _CACHE[key] = _build_nc(repeat)
    return _CACHE[key]


def kernel(x, w_qkv, w_out, b_out):
    from concourse import bass_utils
    nc = get_nc()
    in_maps = _prep_inputs(x, w_qkv, w_out)
    res = bass_utils.run_bass_kernel_spmd(nc, in_maps, core_ids=list(range(NCORES)))
    # out is [e, s] per core -> transpose to [s, e]
    out = np.stack([res.results[i]["out"].T for i in range(NCORES)], axis=0)
    out = out + np.asarray(b_out, dtype=np.float32)[None, None, :]
    return np.ascontiguousarray(out.reshape(B, T, HH, WW, D).astype(np.float32))


# revision 3
# speedup vs baseline: 1.0297x; 1.0297x over previous
"""Fused 2D-RoPE multi-head attention for Trainium2, SPMD over 8 NeuronCores.

Problem: x[2,4,24,24,1024] -> fused QKV -> 16-head attention with 2-axis RoPE
-> out proj.  Data-parallel: one (b t) sequence (S=576, D=1024) per core.

v2 redesign (vs the first working kernel):
  - q/k weight rows are host-permuted into evens-tile/odds-tile PAIRS covering
    4 heads each (tile A = dims {2j} of heads 4g..4g+3, tile B = dims {2j+1}).
    The RoPE rotate-half then pairs partition p of tile A with partition p of
    tile B, so all RoPE arithmetic runs as full-width [128,576] ops (engine
    cost is free-size only; the old layout wasted 3/4 of every lane).
  - RoPE reads the QK-projection PSUM directly (no staging copies through
    the scalar engine); scalar engine only runs exp.
  - Each projection PSUM tile is [128,1024] f32 (2 banks) holding both
    sq-halves at offsets 0/512; exp / reciprocal / normalize / staging then
    process both halves in ONE instruction via a strided AP, halving
    instruction-count on the ACT/DVE engines.
  - Scores contract per head over 32 evens + 32 odds rows (2 accumulating
    matmuls); the 4 heads of a group sit on disjoint 32-row groups of the PE
    array, so their matmuls overlap in the array (row-group tiling).
  - E and the v-with-ones-column table are bf16 (same PE throughput, half
    SBUF/DVE bytes); everything else f32r.
  - Output projection emits [e,s] (oT is already the needed moving operand);
    the host transposes. All weights/inputs are host-packed in exact SBUF
    layout so every DMA is >=0.6MB fully-coalesced (18 DMAs/rep vs 79).
  - Denominator rides as a 65th ones-column in the att@v stationary operand;
    softmax has no max-subtraction (scores ~ N(0,1), exp is safe).
"""

import numpy as np
from contextlib import ExitStack

B, T, HH, WW, D = 2, 4, 24, 24, 1024
NH, HD = 16, 64
S = HH * WW            # 576
BT = B * T             # 8
NCORES = 8
P = 128
SQH = 288              # half of S
NKD = D // P           # 8 contraction tiles over D
S_TILES = [(0, 128), (128, 128), (256, 128), (384, 128), (512, 64)]
VSLOT = HD + 1         # 65: per-head v columns + ones column

_CACHE: dict = {}


def _cs_table():
    """cos/sin tables [128, S]: row 32*i + j = pair-frequency j (any head)."""
    half = HD // 4     # 16
    inv = (1.0 / (10000.0 ** (np.arange(half, dtype=np.float32) / np.float32(half)))).astype(np.float32)
    s = np.arange(S, dtype=np.float32)
    th = (s // WW)[None, :] * inv[:, None]          # [16, S]
    tw = (s % WW)[None, :] * inv[:, None]           # [16, S]
    ang = np.concatenate([th, tw], axis=0)          # [32, S]
    cos32 = np.cos(ang).astype(np.float32)
    sin32 = np.sin(ang).astype(np.float32)
    cos_t = np.tile(cos32, (4, 1))                  # [128, S]
    sin_t = np.tile(sin32, (4, 1))
    return np.ascontiguousarray(np.concatenate([cos_t, sin_t], axis=1))  # [128, 2S]


def _qk_rowperm():
    """Order of the 2048 q+k rows of w_qkv into 16 tiles of 128.

    Tile tt = 4g + r: r=0 kA (k evens), r=1 kB (k odds), r=2 qA, r=3 qB.
    Within a tile, col 32*i + j = head 4g+i, rotation pair j.
    """
    rows = []
    for g in range(4):
        for base, odd in ((D, 0), (D, 1), (0, 0), (0, 1)):
            for i in range(4):
                for j in range(32):
                    rows.append(base + (4 * g + i) * HD + 2 * j + odd)
    return np.asarray(rows)


def _build_nc(repeat=1):
    import concourse.bacc as bacc
    import concourse.mybir as mybir
    from concourse.tile import TileContext

    f32 = mybir.dt.float32
    f32r = mybir.dt.float32r
    bf16 = mybir.dt.bfloat16
    AF = mybir.ActivationFunctionType

    nc = bacc.Bacc("TRN2", target_bir_lowering=False, debug=False)
    xsb_d = nc.dram_tensor("xsb", [P, NKD * S], bf16, kind="ExternalInput").ap()
    wqk_d = nc.dram_tensor("wqk", [P, 16 * NKD * P], bf16, kind="ExternalInput").ap()
    wv_d = nc.dram_tensor("wv", [P, 4 * NKD * 256], bf16, kind="ExternalInput").ap()
    wo_d = nc.dram_tensor("wo", [P, 4 * 2048], f32r, kind="ExternalInput").ap()
    cs_d = nc.dram_tensor("cs", [P, 2 * S], f32, kind="ExternalInput").ap()
    out_d = nc.dram_tensor("out", [D, S], bf16, kind="ExternalOutput").ap()

    def half2(ap):
        # [p, 1024] psum view -> [p, 2, 288] (the written halves at 0 / 512)
        return ap.rearrange("p (c x) -> p c x", x=512)[:, :, 0:SQH]

    def c2(ap):
        # [p, 576] sbuf view -> [p, 2, 288]
        return ap.rearrange("p (c x) -> p c x", c=2)

    with TileContext(nc) as tc, ExitStack() as ctx:
        const = ctx.enter_context(tc.tile_pool(name="const", bufs=1))
        wqkp = ctx.enter_context(tc.tile_pool(name="wqkp", bufs=2))
        wop = ctx.enter_context(tc.tile_pool(name="wop", bufs=2))
        tmpp = ctx.enter_context(tc.tile_pool(name="tmpp", bufs=3))
        ep = ctx.enter_context(tc.tile_pool(name="ep", bufs=6))
        r1p = ctx.enter_context(tc.tile_pool(name="r1p", bufs=2))
        rrp = ctx.enter_context(tc.tile_pool(name="rrp", bufs=2))
        stp = ctx.enter_context(tc.tile_pool(name="stp", bufs=2))
        psum = ctx.enter_context(tc.tile_pool(name="psum", bufs=4, space="PSUM"))

        xsb2 = [const.tile([P, NKD * S], bf16, name=f"xsb{x}") for x in range(2)]
        wv = const.tile([P, 4 * NKD * 256], bf16, name="wv_t")
        cs = const.tile([P, 2 * S], f32, name="cs_t")
        roped = [const.tile([P, 4 * S], f32r, name=f"roped{g}") for g in range(4)]
        va2 = [const.tile([P, 5 * NH * VSLOT], bf16, name=f"va{x}") for x in range(2)]
        oT = [const.tile([P, 4 * S], f32r, name=f"oT{x}") for x in range(2)]

        nc.sync.dma_start(cs, cs_d)
        for x in range(2):
            vv = va2[x].rearrange("p (st h c) -> p st h c", st=5, c=VSLOT)
            nc.gpsimd.memset(vv[:, :, :, HD:HD + 1], 1.0)

        cos_s = c2(cs[:, 0:S])
        sin_s = c2(cs[:, S:2 * S])
        wvv = wv.rearrange("p (c k e) -> p c k e", c=4, e=256)

        for _rep in range(repeat):
            xsb = xsb2[_rep % 2]
            va = va2[_rep % 2]
            vav = va.rearrange("p (st h c) -> p st h c", st=5, c=VSLOT)
            xv = xsb.rearrange("p (k s) -> p k s", k=NKD)
            nc.sync.dma_start(xsb, xsb_d)
            nc.sync.dma_start(wv, wv_d)

            # ---- v projection: psv[s, e], one accumulation group per psum bank
            for ch in range(2):
                for st, (s0, sl) in enumerate(S_TILES):
                    psv = psum.tile([P, 1024], f32, tag="ps", name="psv")
                    for kt in range(NKD):
                        for cc in range(2):
                            nc.tensor.matmul(psv[0:sl, cc * 512:cc * 512 + 256],
                                             xv[:, kt, s0:s0 + sl],
                                             wvv[:, 2 * ch + cc, kt, :],
                                             start=(kt == 0), stop=(kt == NKD - 1))
                    src = psv.rearrange("p (b c h x) -> p b c h x",
                                        b=1, c=2, h=8)[0:sl, :, :, 0:4, :]
                    dst = va.rearrange("p (st c2 h4 x) -> p st c2 h4 x",
                                       st=5, c2=4, x=VSLOT)[0:sl, st:st + 1,
                                                            2 * ch:2 * ch + 2, :, 0:HD]
                    nc.vector.tensor_copy(dst, src)

            # ---- q,k projection + RoPE, per 4-head group g
            for g in range(4):
                wt4 = wqkp.tile([P, 4 * NKD * P], bf16, name="wt4")
                nc.sync.dma_start(wt4, wqk_d[:, g * 4096:(g + 1) * 4096])
                for half in range(2):          # 0: k pair (A,B), 1: q pair
                    wt = wt4[:, half * 2048:(half + 1) * 2048]
                    psA = psum.tile([P, 1024], f32, tag="ps", name="psA")
                    psB = psum.tile([P, 1024], f32, tag="ps", name="psB")
                    for kt in range(NKD):
                        for ti, ps in ((0, psA), (1, psB)):
                            w_r = wt[:, (ti * NKD + kt) * P:(ti * NKD + kt + 1) * P]
                            for hf in range(2):
                                nc.tensor.matmul(ps[:, hf * 512:hf * 512 + SQH],
                                                 w_r, xv[:, kt, hf * SQH:(hf + 1) * SQH],
                                                 start=(kt == 0), stop=(kt == NKD - 1))
                    rA = roped[g][:, (2 * half) * S:(2 * half) * S + S]
                    rB = roped[g][:, (2 * half + 1) * S:(2 * half + 1) * S + S]
                    tA = tmpp.tile([P, S], f32, name="tA")
                    tB = tmpp.tile([P, S], f32, name="tB")
                    nc.vector.tensor_mul(c2(rA), half2(psA), cos_s)
                    nc.vector.tensor_mul(c2(tB), half2(psB), sin_s)
                    nc.vector.tensor_mul(c2(rB), half2(psB), cos_s)
                    nc.vector.tensor_mul(c2(tA), half2(psA), sin_s)
                    nc.gpsimd.tensor_sub(rA, rA, tB)   # evens: a*cos - b*sin
                    nc.gpsimd.tensor_add(rB, rB, tA)   # odds:  b*cos + a*sin

            # ---- attention, per head
            for g in range(4):
                for i in range(4):
                    h = 4 * g + i
                    Es = []
                    for ci, (c0, cl) in enumerate(S_TILES):
                        pss = psum.tile([P, 1024], f32, tag="ps", name="pss")
                        kA = roped[g][32 * i:32 * i + 32, c0:c0 + cl]
                        kB = roped[g][32 * i:32 * i + 32, S + c0:S + c0 + cl]
                        for hf in range(2):
                            qA = roped[g][32 * i:32 * i + 32, 2 * S + hf * SQH:2 * S + (hf + 1) * SQH]
                            nc.tensor.matmul(pss[0:cl, hf * 512:hf * 512 + SQH],
                                             kA, qA, start=True, stop=False,
                                             tile_position=(32 * i, 0))
                        for hf in range(2):
                            qB = roped[g][32 * i:32 * i + 32, 3 * S + hf * SQH:3 * S + (hf + 1) * SQH]
                            nc.tensor.matmul(pss[0:cl, hf * 512:hf * 512 + SQH],
                                             kB, qB, start=False, stop=True,
                                             tile_position=(32 * i, 0))
                        E = ep.tile([P, S], bf16, name="E")
                        nc.scalar.activation(c2(E[0:cl, :]), half2(pss[0:cl, :]),
                                             AF.Exp, scale=0.125)
                        Es.append(E)
                    pso = psum.tile([P, 1024], f32, tag="ps", name="pso")
                    for hf in range(2):
                        for ci, (c0, cl) in enumerate(S_TILES):
                            nc.tensor.matmul(pso[0:VSLOT, hf * 512:hf * 512 + SQH],
                                             vav[0:cl, ci:ci + 1, h:h + 1, :],
                                             Es[ci][0:cl, hf * SQH:(hf + 1) * SQH],
                                             start=(ci == 0), stop=(ci == 4))
                    r1 = r1p.tile([1, S], f32, name="r1")
                    nc.vector.reciprocal(c2(r1), half2(pso[HD:HD + 1, :]))
                    rr = rrp.tile([HD, S], f32, name="rr")
                    nc.gpsimd.partition_broadcast(rr, r1)
                    ot_dst = oT[h // 8][64 * (h % 2):64 * (h % 2) + 64,
                                        ((h // 2) % 4) * S:((h // 2) % 4) * S + S]
                    nc.vector.tensor_mul(
                        ot_dst.rearrange("p (b c x) -> p b c x", b=1, x=SQH),
                        pso.rearrange("p (b c x) -> p b c x", b=1, x=512)[0:HD, :, :, 0:SQH],
                        rr.rearrange("p (b c x) -> p b c x", b=1, x=SQH))

            # ---- output projection: out[e, s]
            for epi in range(4):
                wot = wop.tile([P, 2048], f32r, name="wot")
                nc.sync.dma_start(wot, wo_d[:, epi * 2048:(epi + 1) * 2048])
                ob = stp.tile([P, 2 * S], bf16, name="ob")
                for e2 in range(2):
                    pso2 = psum.tile([P, 1024], f32, tag="ps", name="pso2")
                    for kt in range(NKD):
                        w_r = wot[:, (kt * 2 + e2) * P:(kt * 2 + e2 + 1) * P]
                        mv = oT[kt // 4][:, (kt % 4) * S:(kt % 4) * S + S]
                        for hf in range(2):
                            nc.tensor.matmul(pso2[:, hf * 512:hf * 512 + SQH],
                                             w_r, mv[:, hf * SQH:(hf + 1) * SQH],
                                             start=(kt == 0), stop=(kt == NKD - 1))
                    nc.vector.tensor_copy(c2(ob[:, e2 * S:(e2 + 1) * S]), half2(pso2))
                nc.sync.dma_start(out_d.rearrange("(t p) s -> p t s", p=P)[:, 2 * epi:2 * epi + 2, :],
                                  ob.rearrange("p (t s) -> p t s", t=2))
    nc.compile()
    return nc


def _prep_inputs(x, w_qkv, w_out):
    x = np.asarray(x, dtype=np.float32)
    w_qkv = np.asarray(w_qkv, dtype=np.float32)
    w_out = np.asarray(w_out, dtype=np.float32)
    xr = x.reshape(BT, S, D)

    perm = _qk_rowperm()
    wq_full = w_qkv[perm]                                   # [2048, 1024]
    import ml_dtypes
    bf = ml_dtypes.bfloat16
    wqk_sb = np.ascontiguousarray(
        wq_full.reshape(16, 128, NKD, P).transpose(3, 0, 2, 1).reshape(P, -1).astype(bf))
    wv_arr = w_qkv[2 * D:3 * D]                             # [1024(e), 1024(d)]
    wv_sb = np.ascontiguousarray(
        wv_arr.reshape(4, 256, NKD, P).transpose(3, 0, 2, 1).reshape(P, -1).astype(bf))
    wo_sb = np.ascontiguousarray(
        w_out.reshape(4, 2, P, NKD, P).transpose(4, 0, 3, 1, 2).reshape(P, -1))
    cs = _cs_table()

    in_maps = []
    for i in range(NCORES):
        xsb = np.ascontiguousarray(
            xr[i].T.reshape(NKD, P, S).transpose(1, 0, 2).reshape(P, -1).astype(bf))
        in_maps.append({
            "xsb": xsb, "wqk": wqk_sb, "wv": wv_sb, "wo": wo_sb, "cs": cs,
        })
    return in_maps


def get_nc(repeat=1):
    key = f"nc{repeat}"
    if key not in _CACHE:
        _CACHE[key] = _build_nc(repeat)
    return _CACHE[key]


def kernel(x, w_qkv, w_out, b_out):
    from concourse import bass_utils
    nc = get_nc()
    in_maps = _prep_inputs(x, w_qkv, w_out)
    res = bass_utils.run_bass_kernel_spmd(nc, in_maps, core_ids=list(range(NCORES)))
    # out is [e, s] per core -> transpose to [s, e]
    out = np.stack([res.results[i]["out"].astype(np.float32).T for i in range(NCORES)], axis=0)
    out = out + np.asarray(b_out, dtype=np.float32)[None, None, :]
    return np.ascontiguousarray(out.reshape(B, T, HH, WW, D).astype(np.float32))


# revision 4
# speedup vs baseline: 1.0559x; 1.0255x over previous
"""Fused 2D-RoPE multi-head attention for Trainium2, SPMD over 8 NeuronCores.

Problem: x[2,4,24,24,1024] -> fused QKV -> 16-head attention with 2-axis RoPE
-> out proj.  Data-parallel: one (b t) sequence (S=576, D=1024) per core.

v2 redesign (vs the first working kernel):
  - q/k weight rows are host-permuted into evens-tile/odds-tile PAIRS covering
    4 heads each (tile A = dims {2j} of heads 4g..4g+3, tile B = dims {2j+1}).
    The RoPE rotate-half then pairs partition p of tile A with partition p of
    tile B, so all RoPE arithmetic runs as full-width [128,576] ops (engine
    cost is free-size only; the old layout wasted 3/4 of every lane).
  - RoPE reads the QK-projection PSUM directly (no staging copies through
    the scalar engine); scalar engine only runs exp.
  - Each projection PSUM tile is [128,1024] f32 (2 banks) holding both
    sq-halves at offsets 0/512; exp / reciprocal / normalize / staging then
    process both halves in ONE instruction via a strided AP, halving
    instruction-count on the ACT/DVE engines.
  - Scores contract per head over 32 evens + 32 odds rows (2 accumulating
    matmuls); the 4 heads of a group sit on disjoint 32-row groups of the PE
    array, so their matmuls overlap in the array (row-group tiling).
  - E and the v-with-ones-column table are bf16 (same PE throughput, half
    SBUF/DVE bytes); everything else f32r.
  - Output projection emits [e,s] (oT is already the needed moving operand);
    the host transposes. All weights/inputs are host-packed in exact SBUF
    layout so every DMA is >=0.6MB fully-coalesced (18 DMAs/rep vs 79).
  - Denominator rides as a 65th ones-column in the att@v stationary operand;
    softmax has no max-subtraction (scores ~ N(0,1), exp is safe).
"""

import numpy as np
from contextlib import ExitStack

B, T, HH, WW, D = 2, 4, 24, 24, 1024
NH, HD = 16, 64
S = HH * WW            # 576
BT = B * T             # 8
NCORES = 8
P = 128
SQH = 288              # half of S
NKD = D // P           # 8 contraction tiles over D
S_TILES = [(0, 128), (128, 128), (256, 128), (384, 128), (512, 64)]
VSLOT = HD + 1         # 65: per-head v columns + ones column

_CACHE: dict = {}


def _cs_table():
    """cos/sin tables [128, S]: row 32*i + j = pair-frequency j (any head)."""
    half = HD // 4     # 16
    inv = (1.0 / (10000.0 ** (np.arange(half, dtype=np.float32) / np.float32(half)))).astype(np.float32)
    s = np.arange(S, dtype=np.float32)
    th = (s // WW)[None, :] * inv[:, None]          # [16, S]
    tw = (s % WW)[None, :] * inv[:, None]           # [16, S]
    ang = np.concatenate([th, tw], axis=0)          # [32, S]
    cos32 = np.cos(ang).astype(np.float32)
    sin32 = np.sin(ang).astype(np.float32)
    cos_t = np.tile(cos32, (4, 1))                  # [128, S]
    sin_t = np.tile(sin32, (4, 1))
    return np.ascontiguousarray(np.concatenate([cos_t, sin_t], axis=1))  # [128, 2S]


def _qk_rowperm():
    """Order of the 2048 q+k rows of w_qkv into 16 tiles of 128.

    Tile tt = 4g + r: r=0 kA (k evens), r=1 kB (k odds), r=2 qA, r=3 qB.
    Within a tile, col 32*i + j = head 4g+i, rotation pair j.
    """
    rows = []
    for g in range(4):
        for base, odd in ((D, 0), (D, 1), (0, 0), (0, 1)):
            for i in range(4):
                for j in range(32):
                    rows.append(base + (4 * g + i) * HD + 2 * j + odd)
    return np.asarray(rows)


def _build_nc(repeat=1):
    import concourse.bacc as bacc
    import concourse.mybir as mybir
    from concourse.tile import TileContext

    f32 = mybir.dt.float32
    f32r = mybir.dt.float32r
    bf16 = mybir.dt.bfloat16
    AF = mybir.ActivationFunctionType

    nc = bacc.Bacc("TRN2", target_bir_lowering=False, debug=False)
    xsb_d = nc.dram_tensor("xsb", [P, NKD * S], bf16, kind="ExternalInput").ap()
    wqk_d = nc.dram_tensor("wqk", [P, 16 * NKD * P], bf16, kind="ExternalInput").ap()
    wv_d = nc.dram_tensor("wv", [P, 4 * NKD * 256], bf16, kind="ExternalInput").ap()
    wo_d = nc.dram_tensor("wo", [P, 4 * 2048], f32r, kind="ExternalInput").ap()
    cs_d = nc.dram_tensor("cs", [P, 2 * S], f32, kind="ExternalInput").ap()
    out_d = nc.dram_tensor("out", [D, S], bf16, kind="ExternalOutput").ap()

    def half2(ap):
        # [p, 1024] psum view -> [p, 2, 288] (the written halves at 0 / 512)
        return ap.rearrange("p (c x) -> p c x", x=512)[:, :, 0:SQH]

    def c2(ap):
        # [p, 576] sbuf view -> [p, 2, 288]
        return ap.rearrange("p (c x) -> p c x", c=2)

    with TileContext(nc) as tc, ExitStack() as ctx:
        const = ctx.enter_context(tc.tile_pool(name="const", bufs=1))
        wqkp = ctx.enter_context(tc.tile_pool(name="wqkp", bufs=2))
        wop = ctx.enter_context(tc.tile_pool(name="wop", bufs=2))
        tmpp = ctx.enter_context(tc.tile_pool(name="tmpp", bufs=3))
        ep = ctx.enter_context(tc.tile_pool(name="ep", bufs=6))
        r1p = ctx.enter_context(tc.tile_pool(name="r1p", bufs=2))
        rrp = ctx.enter_context(tc.tile_pool(name="rrp", bufs=2))
        stp = ctx.enter_context(tc.tile_pool(name="stp", bufs=2))
        psum = ctx.enter_context(tc.tile_pool(name="psum", bufs=4, space="PSUM"))

        xsb2 = [const.tile([P, NKD * S], bf16, name=f"xsb{x}") for x in range(2)]
        wv = const.tile([P, 4 * NKD * 256], bf16, name="wv_t")
        cs = const.tile([P, 2 * S], f32, name="cs_t")
        roped = [const.tile([P, 4 * S], f32r, name=f"roped{g}") for g in range(4)]
        va2 = [const.tile([P, 5 * NH * VSLOT], bf16, name=f"va{x}") for x in range(2)]
        oT = [const.tile([P, 4 * S], f32r, name=f"oT{x}") for x in range(2)]

        nc.sync.dma_start(cs, cs_d)
        for x in range(2):
            vv = va2[x].rearrange("p (st h c) -> p st h c", st=5, c=VSLOT)
            nc.gpsimd.memset(vv[:, :, :, HD:HD + 1], 1.0)

        cos_s = c2(cs[:, 0:S])
        sin_s = c2(cs[:, S:2 * S])
        wvv = wv.rearrange("p (c k e) -> p c k e", c=4, e=256)

        for _rep in range(repeat):
            xsb = xsb2[_rep % 2]
            va = va2[_rep % 2]
            vav = va.rearrange("p (st h c) -> p st h c", st=5, c=VSLOT)
            xv = xsb.rearrange("p (k s) -> p k s", k=NKD)
            nc.scalar.dma_start(xsb, xsb_d)
            nc.scalar.dma_start(wv, wv_d)

            # ---- v projection: psv[s, e], one accumulation group per psum bank
            for ch in range(2):
                for st, (s0, sl) in enumerate(S_TILES):
                    psv = psum.tile([P, 1024], f32, tag="ps", name="psv")
                    for kt in range(NKD):
                        for cc in range(2):
                            nc.tensor.matmul(psv[0:sl, cc * 512:cc * 512 + 256],
                                             xv[:, kt, s0:s0 + sl],
                                             wvv[:, 2 * ch + cc, kt, :],
                                             start=(kt == 0), stop=(kt == NKD - 1))
                    src = psv.rearrange("p (b c h x) -> p b c h x",
                                        b=1, c=2, h=8)[0:sl, :, :, 0:4, :]
                    dst = va.rearrange("p (st c2 h4 x) -> p st c2 h4 x",
                                       st=5, c2=4, x=VSLOT)[0:sl, st:st + 1,
                                                            2 * ch:2 * ch + 2, :, 0:HD]
                    nc.vector.tensor_copy(dst, src)

            # ---- q,k projection + RoPE, per 4-head group g
            for g in range(4):
                wt4 = wqkp.tile([P, 4 * NKD * P], bf16, name="wt4")
                nc.scalar.dma_start(wt4, wqk_d[:, g * 4096:(g + 1) * 4096])
                for half in range(2):          # 0: k pair (A,B), 1: q pair
                    wt = wt4[:, half * 2048:(half + 1) * 2048]
                    psA = psum.tile([P, 1024], f32, tag="ps", name="psA")
                    psB = psum.tile([P, 1024], f32, tag="ps", name="psB")
                    for kt in range(NKD):
                        for ti, ps in ((0, psA), (1, psB)):
                            w_r = wt[:, (ti * NKD + kt) * P:(ti * NKD + kt + 1) * P]
                            for hf in range(2):
                                nc.tensor.matmul(ps[:, hf * 512:hf * 512 + SQH],
                                                 w_r, xv[:, kt, hf * SQH:(hf + 1) * SQH],
                                                 start=(kt == 0), stop=(kt == NKD - 1))
                    rA = roped[g][:, (2 * half) * S:(2 * half) * S + S]
                    rB = roped[g][:, (2 * half + 1) * S:(2 * half + 1) * S + S]
                    tA = tmpp.tile([P, S], f32, name="tA")
                    tB = tmpp.tile([P, S], f32, name="tB")
                    nc.vector.tensor_mul(c2(rA), half2(psA), cos_s)
                    nc.vector.tensor_mul(c2(tB), half2(psB), sin_s)
                    nc.vector.tensor_mul(c2(rB), half2(psB), cos_s)
                    nc.vector.tensor_mul(c2(tA), half2(psA), sin_s)
                    nc.gpsimd.tensor_sub(rA, rA, tB)   # evens: a*cos - b*sin
                    nc.gpsimd.tensor_add(rB, rB, tA)   # odds:  b*cos + a*sin

            # ---- attention, per head
            for g in range(4):
                for i in range(4):
                    h = 4 * g + i
                    Es = []
                    for ci, (c0, cl) in enumerate(S_TILES):
                        pss = psum.tile([P, 1024], f32, tag="ps", name="pss")
                        kA = roped[g][32 * i:32 * i + 32, c0:c0 + cl]
                        kB = roped[g][32 * i:32 * i + 32, S + c0:S + c0 + cl]
                        for hf in range(2):
                            qA = roped[g][32 * i:32 * i + 32, 2 * S + hf * SQH:2 * S + (hf + 1) * SQH]
                            nc.tensor.matmul(pss[0:cl, hf * 512:hf * 512 + SQH],
                                             kA, qA, start=True, stop=False,
                                             tile_position=(32 * i, 0))
                        for hf in range(2):
                            qB = roped[g][32 * i:32 * i + 32, 3 * S + hf * SQH:3 * S + (hf + 1) * SQH]
                            nc.tensor.matmul(pss[0:cl, hf * 512:hf * 512 + SQH],
                                             kB, qB, start=False, stop=True,
                                             tile_position=(32 * i, 0))
                        E = ep.tile([P, S], bf16, name="E")
                        nc.scalar.activation(c2(E[0:cl, :]), half2(pss[0:cl, :]),
                                             AF.Exp, scale=0.125)
                        Es.append(E)
                    pso = psum.tile([P, 1024], f32, tag="ps", name="pso")
                    for hf in range(2):
                        for ci, (c0, cl) in enumerate(S_TILES):
                            nc.tensor.matmul(pso[0:VSLOT, hf * 512:hf * 512 + SQH],
                                             vav[0:cl, ci:ci + 1, h:h + 1, :],
                                             Es[ci][0:cl, hf * SQH:(hf + 1) * SQH],
                                             start=(ci == 0), stop=(ci == 4))
                    r1 = r1p.tile([1, S], f32, name="r1")
                    nc.vector.reciprocal(c2(r1), half2(pso[HD:HD + 1, :]))
                    rr = rrp.tile([HD, S], f32, name="rr")
                    nc.gpsimd.partition_broadcast(rr, r1)
                    ot_dst = oT[h // 8][64 * (h % 2):64 * (h % 2) + 64,
                                        ((h // 2) % 4) * S:((h // 2) % 4) * S + S]
                    nc.vector.tensor_mul(
                        ot_dst.rearrange("p (b c x) -> p b c x", b=1, x=SQH),
                        pso.rearrange("p (b c x) -> p b c x", b=1, x=512)[0:HD, :, :, 0:SQH],
                        rr.rearrange("p (b c x) -> p b c x", b=1, x=SQH))

            # ---- output projection: out[e, s]
            for ep2 in range(2):
                wot2 = wop.tile([P, 4096], f32r, name="wot2")
                nc.sync.dma_start(wot2, wo_d[:, ep2 * 4096:(ep2 + 1) * 4096])
                ob2 = stp.tile([P, 4 * S], bf16, name="ob2")
                for epi_h in range(2):
                    epi = 2 * ep2 + epi_h
                    wot = wot2[:, epi_h * 2048:(epi_h + 1) * 2048]
                    for e2 in range(2):
                        pso2 = psum.tile([P, 1024], f32, tag="ps", name="pso2")
                        for kt in range(NKD):
                            w_r = wot[:, (kt * 2 + e2) * P:(kt * 2 + e2 + 1) * P]
                            mv = oT[kt // 4][:, (kt % 4) * S:(kt % 4) * S + S]
                            for hf in range(2):
                                nc.tensor.matmul(pso2[:, hf * 512:hf * 512 + SQH],
                                                 w_r, mv[:, hf * SQH:(hf + 1) * SQH],
                                                 start=(kt == 0), stop=(kt == NKD - 1))
                        nc.vector.tensor_copy(
                            c2(ob2[:, (2 * epi_h + e2) * S:(2 * epi_h + e2 + 1) * S]),
                            half2(pso2))
                nc.sync.dma_start(out_d.rearrange("(t p) s -> p t s", p=P)[:, 4 * ep2:4 * ep2 + 4, :],
                                  ob2.rearrange("p (t s) -> p t s", t=4))
    nc.compile()
    return nc


def _prep_inputs(x, w_qkv, w_out):
    x = np.asarray(x, dtype=np.float32)
    w_qkv = np.asarray(w_qkv, dtype=np.float32)
    w_out = np.asarray(w_out, dtype=np.float32)
    xr = x.reshape(BT, S, D)

    perm = _qk_rowperm()
    wq_full = w_qkv[perm]                                   # [2048, 1024]
    import ml_dtypes
    bf = ml_dtypes.bfloat16
    wqk_sb = np.ascontiguousarray(
        wq_full.reshape(16, 128, NKD, P).transpose(3, 0, 2, 1).reshape(P, -1).astype(bf))
    wv_arr = w_qkv[2 * D:3 * D]                             # [1024(e), 1024(d)]
    wv_sb = np.ascontiguousarray(
        wv_arr.reshape(4, 256, NKD, P).transpose(3, 0, 2, 1).reshape(P, -1).astype(bf))
    wo_sb = np.ascontiguousarray(
        w_out.reshape(4, 2, P, NKD, P).transpose(4, 0, 3, 1, 2).reshape(P, -1))
    cs = _cs_table()

    in_maps = []
    for i in range(NCORES):
        xsb = np.ascontiguousarray(
            xr[i].T.reshape(NKD, P, S).transpose(1, 0, 2).reshape(P, -1).astype(bf))
        in_maps.append({
            "xsb": xsb, "wqk": wqk_sb, "wv": wv_sb, "wo": wo_sb, "cs": cs,
        })
    return in_maps


def get_nc(repeat=1):
    key = f"nc{repeat}"
    if key not in _CACHE:
        _CACHE[key] = _build_nc(repeat)
    return _CACHE[key]


def kernel(x, w_qkv, w_out, b_out):
    from concourse import bass_utils
    nc = get_nc()
    in_maps = _prep_inputs(x, w_qkv, w_out)
    res = bass_utils.run_bass_kernel_spmd(nc, in_maps, core_ids=list(range(NCORES)))
    # out is [e, s] per core -> transpose to [s, e]
    out = np.stack([res.results[i]["out"].astype(np.float32).T for i in range(NCORES)], axis=0)
    out = out + np.asarray(b_out, dtype=np.float32)[None, None, :]
    return np.ascontiguousarray(out.reshape(B, T, HH, WW, D).astype(np.float32))
